# revision 4
# baseline (speedup 1.0000x reference)
"""Trainium2 Bass kernel for SKalmanNet GSS (dense GEMV chain, batch=1).

Strategy (8 NeuronCores):
  - The two branches (Pk from l1/gru1/l2, Sk from l3/gru2/l4) are independent
    and have identical shapes -> one SPMD program, cores 0-3 run branch P,
    cores 4-7 run branch S (replica_groups [[0,1,2,3],[4,5,6,7]]).
  - Within a group of 4 cores: tensor-parallel row-sharding of every weight
    matrix; AllGather of the small activation vector after l1, after the GRU
    cell, and after l2_W1.  The final l2_W2 output shard goes straight to the
    per-core output (host concatenates).
  - Each matvec is computed with the activation chunk as the *stationary*
    matmul operand ([128,1]->bcast 32) and the (host-pre-transposed) weight
    tile as the *moving* operand so the TensorEngine streams weights at one
    128-row column per cycle per chain; 2-4 column-group chains run
    concurrently.  PSUM accumulates over input chunks.
  - Weight dtypes are mixed for HBM-bandwidth: l1/Wih/Whh stream as
    float8e3 (e3m4, pre-scaled x64 on host; descaled in the chain-reduce
    constant), W1/W2 stay fp16 (their quantization error lands directly on
    the output).  Activations are fp16 (mixed-dtype matmul).
  - A dummy 8-byte AllGather is issued at kernel start so the one-time
    collectives barrier / cross-rank skew (~37us) is absorbed while the
    weight stream warms up, instead of stalling the first real AllGather.
"""

import os

import numpy as np

X = 32
Y = 32
H1 = 5120          # l1 rows
HID = 2048         # gru hidden
H2 = 4096          # l2_W1 rows
OUT = 1024         # l2_W2 rows (X*X)
IN = 1120          # input vec (2X + Y + XY)
INP = 1152         # padded to 9*128 (slot 1120 = 1.0 for folded l1 bias)

NCORES = 8
TP = 4             # cores per branch

# per-core shard sizes
M_L1 = H1 // TP        # 1280
M_G = 3 * (HID // TP)  # 1536  (r|z|n gate rows, 512 each)
M_W1 = H2 // TP        # 1024
M_W2 = OUT // TP       # 256
HSH = HID // TP        # 512

# (K chunks of 128 input dims, chunks per DMA group, output cols per chunk)
L_L1 = (INP // 128, 9, M_L1)    # (9, 9, 1280)  -> dram [1, 128, 11520]
L_IH = (H1 // 128, 4, M_G)      # (40, 4, 1536) -> [10, 128, 6144]
L_HH = (HID // 128, 4, M_G)     # (16, 4, 1536) -> [4, 128, 6144]
L_W1 = (HID // 128, 4, M_W1)    # (16, 4, 1024) -> [4, 128, 4096]
L_W2 = (H2 // 128, 16, M_W2)    # (32, 16, 256) -> [2, 128, 4096]

# consts layout (f32, [1, 3840]):
#   b_rz(1024) | bih_n(512) | bhh_n(512) | b1(1024) | b2(256) | h_shard(512)
C_BRZ = 0
C_BIHN = 1024
C_BHHN = 1536
C_B1 = 2048
C_B2 = 3072
C_HSH = 3328
C_TOT = 3840

# fp8 (e3m4) weight pre-scale: weights are ~N(0, 0.02); x64 puts them in
# e3m4's normal range (tiny=0.25, max=15.5).  Descaled in the chain-reduce.
W8SCALE = 64.0

# which weight tensors stream as fp8 e3m4 (rest fp16)
FP8_SET = set(os.environ.get("KERNEL_FP8", "wl1,wih,whh").split(","))
WARMCC = os.environ.get("KERNEL_WARMCC", "1") == "1"
WBUFS = int(os.environ.get("KERNEL_WBUFS", "14"))

_CACHE = {}


def _build_nc():
    import concourse.bass as bass  # noqa: F401
    import concourse.mybir as mybir
    import concourse.tile as tile
    from concourse import bacc

    f32 = mybir.dt.float32
    f16 = mybir.dt.float16
    e3 = mybir.dt.float8e3

    def wdt_of(name):
        return e3 if name in FP8_SET else f16

    nc = bacc.Bacc("TRN2", target_bir_lowering=False, debug=False,
                   num_devices=NCORES)

    # x / hn arrive pre-chunked ([p, k] with element k*128+p at [p, k]) so
    # the SBUF load is a plain 2D copy instead of a 2-byte-per-partition
    # scatter.
    x_d = nc.dram_tensor("x", [128, INP // 128], f16, kind="ExternalInput")
    hn_d = nc.dram_tensor("hn", [128, HID // 128], f16, kind="ExternalInput")
    consts_d = nc.dram_tensor("consts", [1, C_TOT], f32, kind="ExternalInput")

    def wtensor(name, spec):
        K, G, Ms = spec
        return nc.dram_tensor(name, [K // G, 128, G * Ms], wdt_of(name),
                              kind="ExternalInput")

    wl1_d = wtensor("wl1", L_L1)
    wih_d = wtensor("wih", L_IH)
    whh_d = wtensor("whh", L_HH)
    w1_d = wtensor("w1", L_W1)
    w2_d = wtensor("w2", L_W2)
    out_d = nc.dram_tensor("out", [1, M_W2], f32, kind="ExternalOutput")

    AF = mybir.ActivationFunctionType
    groups = [[0, 1, 2, 3], [4, 5, 6, 7]]
    agc = [0]  # unique-name counter for collective bounce tiles

    with tile.TileContext(nc) as tc, \
         tc.tile_pool(name="w", bufs=WBUFS) as wpool, \
         tc.tile_pool(name="wl1p", bufs=1) as wl1pool, \
         tc.tile_pool(name="act", bufs=1) as apool, \
         tc.tile_pool(name="ps", bufs=8, space="PSUM") as ppool, \
         tc.tile_pool(name="dram", bufs=1, space="DRAM") as dpool:

        consts = apool.tile([1, C_TOT], f32, tag="consts", name="consts_sb")
        nc.gpsimd.dma_start(consts, consts_d.ap())
        x0 = apool.tile([128, INP // 128], f16, tag="x0", name="x0")
        nc.gpsimd.dma_start(x0, x_d.ap())
        hx = apool.tile([128, HID // 128], f16, tag="hx", name="hx")
        nc.gpsimd.dma_start(hx, hn_d.ap())

        # ---- warm-up collective: absorbs the one-time CC barrier and any
        #      cross-rank start skew while the weight DMA stream spins up.
        if WARMCC:
            wcc = apool.tile([1, 2], f32, tag="wcc", name="wcc")
            nc.vector.memset(wcc, 0.0)
            wi = dpool.tile([1, 2], f32, tag="wcci", name="wcc_in")
            wo = dpool.tile([TP, 2], f32, tag="wcco", name="wcc_out")
            nc.scalar.dma_start(wi, wcc)
            nc.gpsimd.collective_compute(
                "AllGather", mybir.AluOpType.bypass,
                replica_groups=groups,
                ins=[wi.opt()], outs=[wo.opt()],
            )

        # Preload ACT LUTs so sigmoid/tanh don't pay table-load latency on
        # the critical path.
        warm = apool.tile([1, 32], f32, tag="warm", name="warm")
        nc.vector.memset(warm, 0.0)
        nc.scalar.activation(warm, warm, AF.Sigmoid)
        nc.scalar.activation(warm, warm, AF.Tanh)
        nc.scalar.activation(warm, warm, AF.Relu)

        # cross-chain reduction operand: every copied psum row holds its
        # chain's full sum (broadcast-32 stationary), so summing the first
        # 32*nch racc rows with weight 1/32 yields the chain total exactly.
        # sel8 additionally folds the 1/W8SCALE fp8 weight descale.
        sel16 = apool.tile([128, 1], f16, tag="sel16", name="sel16")
        nc.vector.memset(sel16, 1.0 / 32.0)
        sel8 = apool.tile([128, 1], f16, tag="sel8", name="sel8")
        nc.vector.memset(sel8, 1.0 / (32.0 * W8SCALE))

        def gemv(x_sb, wt_d, spec, wname, xmap=None, nch=2, pool=None):
            """psum tiles [1,<=512] = W_shard @ x.

            The K input chunks are split round-robin over `nch` PE
            column-group chains (each chain accumulates in its own PSUM
            bank at a distinct quadrant via tile_position) so the chains
            stream through the array concurrently.  The stationary
            x-column is broadcast to 32 array columns (all 32 psum rows of
            a quadrant hold the chain sum), and a final 1-column matmul
            with sel adds one row per chain into the [1,mw] result.
            nch is bounded by PSUM banks: nm*nch + transients <= 8.
            """
            K, G, Ms = spec
            wdt = wdt_of(wname)
            sel = sel8 if wname in FP8_SET else sel16
            nm = (Ms + 511) // 512
            mts = [(i * 512, min(512, Ms - i * 512)) for i in range(nm)]
            ci = agc[0]
            agc[0] += 1
            # accs[mi][c] — one psum bank per (m-tile, chain); chain c
            # occupies quadrant rows 32c..32c+32
            accs = [[ppool.tile([128, mw], f32, tag="ps",
                                name=f"acc{ci}_{i}_{c}")
                     for c in range(nch)]
                    for i, (_, mw) in enumerate(mts)]
            last_k = [max(k for k in range(K) if k % nch == c)
                      for c in range(nch)]
            for g in range(K // G):
                wt = (pool or wpool).tile([128, G * Ms], wdt, tag="w",
                                          name="wt")
                # weights stream on the SP HWDGE ring only; the ACT ring is
                # kept clear for latency-critical bounce/transpose DMAs
                nc.sync.dma_start(wt, wt_d.ap()[g])
                for j in range(G):
                    k = g * G + j
                    kk = xmap(k) if xmap else k
                    c = k % nch
                    r0 = 32 * c
                    xbc = x_sb[:, kk:kk + 1].broadcast_to([128, 32])
                    for mi, (m0, mw) in enumerate(mts):
                        nc.tensor.matmul(
                            accs[mi][c][r0:r0 + 32, :],
                            xbc,
                            wt[:, j * Ms + m0: j * Ms + m0 + mw],
                            start=(k == c), stop=(k == last_k[c]),
                            tile_position=(0, r0),
                        )
            ps = []
            for mi, (m0, mw) in enumerate(mts):
                racc = apool.tile([128, 512], f16, tag="racc", bufs=3,
                                  name=f"racc{ci}_{mi}")
                for c in range(nch):
                    r0 = 32 * c
                    nc.scalar.copy(racc[r0:r0 + 32, :mw],
                                   accs[mi][c][r0:r0 + 32, :])
                op = ppool.tile([1, mw], f32, tag="ps", name=f"pso{ci}_{mi}")
                nc.tensor.matmul(op[:, :], sel[:32 * nch, :],
                                 racc[:32 * nch, :mw],
                                 start=True, stop=True)
                ps.append(op)
            return ps, mts

        def allgather(y_sb, n, n_pad, nm_name):
            """y_sb [1,n_pad] shard (cols n..n_pad zero) -> SBUF [128, 4*n_pad/128]
            chunk-layout gathered vector (per-rank tail chunks zero)."""
            i = agc[0]
            agc[0] += 1
            agin = dpool.tile([1, n_pad], y_sb.dtype, tag=f"agi{i}",
                              name=f"agin_{nm_name}")
            agout = dpool.tile([TP, n_pad], y_sb.dtype, tag=f"ago{i}",
                               name=f"agout_{nm_name}")
            nc.scalar.dma_start(agin, y_sb)
            nc.gpsimd.collective_compute(
                "AllGather", mybir.AluOpType.bypass,
                replica_groups=groups,
                ins=[agin.opt()], outs=[agout.opt()],
            )
            kt = TP * n_pad // 128
            xt = apool.tile([128, kt], y_sb.dtype,
                            tag=f"x{i}", name=f"x_{nm_name}")
            # hardware X-bar transpose: DRAM [kt,128] -> SBUF [128,kt]
            nc.scalar.dma_start_transpose(
                xt, agout.rearrange("r (k p) -> (r k) p", p=128))
            return xt

        # ---- l1 first: it feeds the first AllGather (critical path)
        #      (bias folded into row 1120 of wl1)
        ps1, mts1 = gemv(x0, wl1_d, L_L1, "wl1", pool=wl1pool)
        # ---- GRU hidden-side matvec: depends only on hn; its matmuls fill
        #      the PE while the l1 AllGather is in flight
        gh, _ = gemv(hx, whh_d, L_HH, "whh")     # 3x [1,512] (r,z,n)
        # stash gh in SBUF: frees its psum banks before gi's chains open,
        # and lets the gate math read it as the second DVE operand
        ghs = apool.tile([1, 1536], f32, tag="ghs", name="ghs")
        for gg in range(3):
            nc.scalar.copy(ghs[:, gg * 512:(gg + 1) * 512], gh[gg][:, :])

        # padded AG staging (pad region zeroed once, off critical path)
        PL1 = 1536
        xmap1 = lambda k: (k // 10) * (PL1 // 128) + k % 10   # noqa: E731
        xmap3 = lambda k: (k // 8) * (PL1 // 128) + k % 8     # noqa: E731
        y1 = apool.tile([1, PL1], f16, tag="y1", name="y1")
        nc.vector.memset(y1[:, M_L1:PL1], 0.0)
        for mi, (m0, mw) in enumerate(mts1):
            nc.scalar.activation(y1[:, m0:m0 + mw], ps1[mi][:, :], AF.Relu)
        x1 = allgather(y1, M_L1, PL1, "l1")      # [128, 48]

        # ---- GRU input-side matvec
        gi, _ = gemv(x1, wih_d, L_IH, "wih", xmap=xmap1)  # 3x [1,512]

        # ---- GRU cell elementwise (shard of 512 hidden units)
        brz = consts[:, C_BRZ:C_BRZ + 1024]
        bihn = consts[:, C_BIHN:C_BIHN + 512]
        bhhn = consts[:, C_BHHN:C_BHHN + 512]
        hsh = consts[:, C_HSH:C_HSH + 512]

        t_r = apool.tile([1, 512], f32, tag="t_r", name="t_r")
        nc.vector.tensor_add(t_r, gi[0][:, :], brz[:, 0:512])
        nc.vector.tensor_add(t_r, t_r, ghs[:, 0:512])
        nc.scalar.activation(t_r, t_r, AF.Sigmoid)          # r

        t_z = apool.tile([1, 512], f32, tag="t_z", name="t_z")
        nc.vector.tensor_add(t_z, gi[1][:, :], brz[:, 512:1024])
        nc.vector.tensor_add(t_z, t_z, ghs[:, 512:1024])
        nc.scalar.activation(t_z, t_z, AF.Sigmoid)          # z

        t_hn = apool.tile([1, 512], f32, tag="t_hn", name="t_hn")
        nc.vector.tensor_add(t_hn, ghs[:, 1024:1536], bhhn)       # hn-gate pre
        t_n = apool.tile([1, 512], f32, tag="t_n", name="t_n")
        nc.vector.tensor_add(t_n, gi[2][:, :], bihn)        # in-gate pre
        nc.vector.tensor_mul(t_hn, t_r, t_hn)               # r * hn
        nc.vector.tensor_add(t_n, t_n, t_hn)
        nc.scalar.activation(t_n, t_n, AF.Tanh)             # n

        t_d = apool.tile([1, 512], f32, tag="t_d", name="t_d")
        nc.vector.tensor_sub(t_d, hsh, t_n)                 # h - n
        nc.vector.tensor_mul(t_d, t_z, t_d)                 # z*(h-n)
        hq = apool.tile([1, HSH], f16, tag="hq", name="hq")
        nc.vector.tensor_add(hq, t_n, t_d)                  # h' = n + z*(h-n)

        x2 = allgather(hq, HSH, HSH, "gru")      # [128, 16]

        # ---- l2_W1: relu(W1 @ h' + b1)
        ps3, mts3 = gemv(x2, w1_d, L_W1, "w1", nch=3)
        y3 = apool.tile([1, PL1], f16, tag="y3", name="y3")
        nc.vector.memset(y3[:, M_W1:PL1], 0.0)
        for mi, (m0, mw) in enumerate(mts3):
            nc.vector.tensor_add(y3[:, m0:m0 + mw], ps3[mi][:, :],
                                 consts[:, C_B1 + m0:C_B1 + m0 + mw])
            nc.scalar.activation(y3[:, m0:m0 + mw], y3[:, m0:m0 + mw],
                                 AF.Relu)
        x3 = allgather(y3, M_W1, PL1, "w1")      # [128, 48]

        # ---- l2_W2: W2 @ y3 + b2  -> per-core output shard
        ps4, _ = gemv(x3, w2_d, L_W2, "w2", xmap=xmap3, nch=4)  # 1x [1,256]
        yo = apool.tile([1, M_W2], f32, tag="yo", name="yo")
        nc.vector.tensor_add(yo, ps4[0][:, :], consts[:, C_B2:C_B2 + M_W2])
        nc.gpsimd.dma_start(out_d.ap(), yo)

    nc.finalize()
    return nc


def _pack(wt, K, G, Ms):
    """[K*128, Ms] input-dim-major transposed weight -> [K//G, 128, G*Ms]."""
    return np.ascontiguousarray(
        wt.reshape(K // G, G, 128, Ms).transpose(0, 2, 1, 3)
        .reshape(K // G, 128, G * Ms))


def _np_wdt(name):
    import ml_dtypes
    if name in FP8_SET:
        return ml_dtypes.float8_e3m4
    return np.float16


def _wcast(w, name):
    if name in FP8_SET:
        return (w * W8SCALE).astype(_np_wdt(name))
    return w.astype(np.float16)


def _prep_core(r, xvec, hn, l1W, l1b, Wih, Whh, bih, bhh, W1, b1, W2, b2):
    f32 = np.float32

    rs = slice(r * M_L1, (r + 1) * M_L1)
    wt = np.zeros((INP, M_L1), f32)
    wt[:IN] = l1W[rs].T
    wt[IN] = l1b[rs]
    wl1 = _pack(wt, *L_L1)

    gsl = [slice(g * HID + r * HSH, g * HID + (r + 1) * HSH) for g in range(3)]
    gidx = np.concatenate([np.arange(s.start, s.stop) for s in gsl])
    wih = _pack(np.ascontiguousarray(Wih[gidx].T), *L_IH)
    whh = _pack(np.ascontiguousarray(Whh[gidx].T), *L_HH)
    w1 = _pack(np.ascontiguousarray(W1[r * M_W1:(r + 1) * M_W1].T), *L_W1)
    w2 = _pack(np.ascontiguousarray(W2[r * M_W2:(r + 1) * M_W2].T), *L_W2)

    bsum = bih + bhh
    consts = np.concatenate([
        bsum[gsl[0]], bsum[gsl[1]],          # b_rz
        bih[gsl[2]], bhh[gsl[2]],            # bih_n, bhh_n
        b1[r * M_W1:(r + 1) * M_W1],
        b2[r * M_W2:(r + 1) * M_W2],
        hn[r * HSH:(r + 1) * HSH],
    ]).astype(f32)[None]
    assert consts.shape[1] == C_TOT

    x = np.zeros(INP, f32)
    x[:IN] = xvec
    x[IN] = 1.0
    x_ch = np.ascontiguousarray(x.reshape(INP // 128, 128).T)
    hn_ch = np.ascontiguousarray(hn.reshape(HID // 128, 128).T)

    return {
        "x": x_ch.astype(np.float16), "hn": hn_ch.astype(np.float16),
        "consts": consts,
        "wl1": _wcast(wl1, "wl1"), "wih": _wcast(wih, "wih"),
        "whh": _wcast(whh, "whh"), "w1": _wcast(w1, "w1"),
        "w2": _wcast(w2, "w2"),
    }


LAST_RESULT = None
WDT_NAME = "mixed-fp8"


def kernel(state_inno, observation_inno, diff_state, diff_obs,
           linearization_error, Jacobian,
           l1_W, l1_b, gru1_Wih, gru1_Whh, gru1_bih, gru1_bhh,
           l2_W1, l2_b1, l2_W2, l2_b2,
           l3_W, l3_b, gru2_Wih, gru2_Whh, gru2_bih, gru2_bhh,
           l4_W1, l4_b1, l4_W2, l4_b2, hn1, hn2):
    global LAST_RESULT
    from concourse.bass_utils import run_bass_kernel_spmd

    if "nc" not in _CACHE:
        _CACHE["nc"] = _build_nc()
    nc = _CACHE["nc"]

    a = lambda v: np.asarray(v, dtype=np.float32)
    input1 = np.concatenate([a(state_inno), a(diff_state),
                             a(linearization_error), a(Jacobian)]).reshape(-1)
    input2 = np.concatenate([a(observation_inno), a(diff_obs),
                             a(linearization_error), a(Jacobian)]).reshape(-1)

    branches = [
        (input1, a(hn1).reshape(-1), a(l1_W), a(l1_b).reshape(-1),
         a(gru1_Wih), a(gru1_Whh), a(gru1_bih).reshape(-1),
         a(gru1_bhh).reshape(-1), a(l2_W1), a(l2_b1).reshape(-1),
         a(l2_W2), a(l2_b2).reshape(-1)),
        (input2, a(hn2).reshape(-1), a(l3_W), a(l3_b).reshape(-1),
         a(gru2_Wih), a(gru2_Whh), a(gru2_bih).reshape(-1),
         a(gru2_bhh).reshape(-1), a(l4_W1), a(l4_b1).reshape(-1),
         a(l4_W2), a(l4_b2).reshape(-1)),
    ]
    in_maps = [_prep_core(c % TP, *branches[c // TP]) for c in range(NCORES)]

    kwargs = {}
    if os.environ.get("KERNEL_TRACE"):
        cores = os.environ.get("KERNEL_TRACE_CORES", "0")
        kwargs.update(trace=True,
                      trace_cores=[int(c) for c in cores.split(",")])

    res = run_bass_kernel_spmd(nc, in_maps, core_ids=list(range(NCORES)),
                               **kwargs)
    LAST_RESULT = res
    outs = [res.results[c]["out"].reshape(-1) for c in range(NCORES)]
    Pk = np.concatenate(outs[:TP]).reshape(X, X).astype(np.float32)
    Sk = np.concatenate(outs[TP:]).reshape(Y, Y).astype(np.float32)
    return Pk, Sk
